# revision 24
# baseline (speedup 1.0000x reference)
"""Trainium2 Bass kernel for the binarized CNN:
conv3x3(sign weights) -> BN -> ternary hardtanh -> maxpool4 -> linear(sign weights)

Strategy (pure data parallel over batch, 8 cores x 512 samples):
  - Conv as K~116 matmuls with EXACT bf16 operands: x is split hi/lo into two
    bf16 planes on the host (products bf16*{-1,0,1} and bf16 bias rows are
    exact; fp32 PSUM accumulation), so the conv matches the reference within
    the certified ternary-threshold margins while streaming at 2 (not fp32's
    4) PE cycles per output column. Per (batch-tile, row): 6 matmuls
    (2 passes x 3 PSUM banks). Host also builds the full im2col matrix, so
    no on-device transposes or SBUF gather DMAs are needed.
  - BN + ternary thresholds fold into per-channel fp32 thresholds: the conv
    emits Z = sign-conv(x) - tau_mid[c] via two bf16 bias rows; ternary
    t' = (Z>dH[c]) + (Z>dL[c]) in {0,1,2} via tensor-tensor compares
    against replicated threshold tiles; the -1 shift folds into fc bias.
  - maxpool commutes with the monotone BN+ternary: w-pool runs as a strided
    reduce_max straight from PSUM (phase-slab weight column order,
    col = ww*288 + g), h-pool as tensor_max over the 4 row tiles.
  - FC: DMA-transpose t' tiles to [feature, batch], 9 accumulating bf16
    matmuls; output written [10, 512], un-transposed on host.
  NOTE: an int16-quantized pooling variant (ScalarE saturating-cast
  evacuation + 2x-rate int16 DVE pool) was ~same speed but showed rare
  nondeterministic corruption on hardware; this fp32 pool path is stable.
"""

import numpy as np
from contextlib import ExitStack

import concourse.bass as bass
import concourse.tile as tile
from concourse import bacc, mybir
from concourse.bass_utils import run_bass_kernel_spmd

import os as _os

F32 = mybir.dt.float32
BF16 = mybir.dt.bfloat16
I16 = mybir.dt.int16
ALU = mybir.AluOpType

# Walrus LDWEIGHTS dedup: crashes codegen (visitInstLdweights) on this
# kernel, so off by default; kept behind an env flag for experiments.
if _os.environ.get("KLDW", "0") == "1":
    from concourse import bass_utils as _bu
    if not getattr(_bu, "_ldw_cmd_patched", False):
        _orig_run_command = _bu.run_command

        def _run_command(cmd, *a, **k):
            cmd = [x if x != "--enable-ldw-opt=false" else "--enable-ldw-opt=true"
                   for x in cmd]
            return _orig_run_command(cmd, *a, **k)

        _bu.run_command = _run_command
        _bu._ldw_cmd_patched = True

NCORES = 8
BFULL = 4096
B = BFULL // NCORES          # 512 per core
P = 128
BT = B // P                  # 4 batch tiles
H, W = 14, 38
HO, WO = 12, 36
C = 32
KH = 116                     # 114 patch rows + 2 bias rows (hi plane)
KL = 114                     # lo plane: patch rows only
NF = C * WO                  # 1152 conv outputs per (b, h)
CW3 = C * (WO // 4)          # 288 after w-pool
EPS = 1e-5
NOUT = 10
NK = BT * HO                 # 48 conv tiles
QS = 16384.0                 # int16 quant scale 2^14


def _host_prep(conv_w, conv_b, bn_gamma, bn_beta, bn_mean, bn_var, fc_w, fc_b):
    import ml_dtypes
    f64 = np.float64
    inv = bn_gamma.astype(f64) / np.sqrt(bn_var.astype(f64) + EPS)
    tauH = (0.5 - bn_beta) / inv + bn_mean - conv_b      # y>0.5  <=> z>tauH
    tauL = (-0.5 - bn_beta) / inv + bn_mean - conv_b
    tmid = 0.5 * (tauH + tauL)
    bh = (-tmid).astype(np.float32).astype(ml_dtypes.bfloat16)
    bl = ((-tmid) - bh.astype(f64)).astype(np.float32).astype(ml_dtypes.bfloat16)
    dH = (tauH - tmid).astype(np.float32)
    dL = (tauL - tmid).astype(np.float32)

    sw = np.sign(conv_w[:, 0]).astype(np.float32)        # [32, 3, 3]
    wt = np.zeros((KH, NF), np.float32)
    for c in range(C):
        for w3 in range(WO // 4):
            for ww in range(4):
                w = 4 * w3 + ww
                n = ww * CW3 + c * 9 + w3                # phase-slab order
                for i in range(3):
                    for j in range(3):
                        wt[i * W + w + j, n] = sw[c, i, j]
                wt[114, n] = bh[c]
                wt[115, n] = bl[c]

    thr = np.zeros((P, 2 * CW3), np.float32)
    for c in range(C):
        for w3 in range(9):
            thr[:, c * 9 + w3] = dH[c]
            thr[:, CW3 + c * 9 + w3] = dL[c]

    sf = np.sign(fc_w).astype(np.float32)                # [10, 864]
    sfc = np.zeros((P, 9 * NOUT), np.float32)
    for jj in range(9):
        h3, ch = jj // 3, jj % 3
        kj = 32 if ch == 2 else 128
        for r in range(kj):
            rg = ch * 128 + r                            # index into (c, w3)
            c, w3 = rg // 9, rg % 9
            f = c * 27 + h3 * 9 + w3                     # reference flatten order
            sfc[r, jj * NOUT:(jj + 1) * NOUT] = sf[:, f]

    fcb = (fc_b.astype(f64) - sf.astype(f64).sum(axis=1)).astype(np.float32)
    return (wt.astype(ml_dtypes.bfloat16), thr,
            sfc.astype(ml_dtypes.bfloat16), fcb.reshape(NOUT, 1))


def _host_im2col(xc):
    """xc [512, 532] f32 -> (imh [116, 6144] bf16, iml [114, 6144] bf16),
    columns ordered (bt, h, b)."""
    import ml_dtypes
    xh = xc.astype(ml_dtypes.bfloat16)
    xl = (xc - xh.astype(np.float32)).astype(ml_dtypes.bfloat16)

    def cols(a):
        win = np.lib.stride_tricks.sliding_window_view(a, 114, axis=1)[:, ::W]
        return win.reshape(BT, P, HO, 114).transpose(3, 0, 2, 1).reshape(114, -1)

    imh = np.empty((KH, NK * P), ml_dtypes.bfloat16)
    imh[:114] = cols(xh)
    imh[114:] = 1.0
    iml = np.ascontiguousarray(cols(xl))
    return imh, iml


def _build():
    nc = bacc.Bacc("TRN2", target_bir_lowering=False, debug=False,
                   num_devices=NCORES)
    imh_d = nc.dram_tensor("imh", [KH, NK * P], BF16, kind="ExternalInput").ap()
    iml_d = nc.dram_tensor("iml", [KL, NK * P], BF16, kind="ExternalInput").ap()
    wt_d = nc.dram_tensor("wt", [KH, NF], BF16, kind="ExternalInput").ap()
    thr_d = nc.dram_tensor("thr", [P, 2 * CW3], F32, kind="ExternalInput").ap()
    sfc_d = nc.dram_tensor("sfc", [P, 9 * NOUT], BF16, kind="ExternalInput").ap()
    fcb_d = nc.dram_tensor("fcb", [NOUT, 1], F32, kind="ExternalInput").ap()
    out_d = nc.dram_tensor("out", [NOUT, B], F32, kind="ExternalOutput").ap()

    with tile.TileContext(nc) as tc, ExitStack() as ctx:
        const = ctx.enter_context(tc.tile_pool(name="const", bufs=1))
        imp = ctx.enter_context(tc.tile_pool(name="imp", bufs=1))
        zqp = ctx.enter_context(tc.tile_pool(name="zq", bufs=7))
        yp = ctx.enter_context(tc.tile_pool(name="y", bufs=6))
        gp = ctx.enter_context(tc.tile_pool(name="g", bufs=6))
        ttp = ctx.enter_context(tc.tile_pool(name="tt", bufs=1))

        wt = const.tile([KH, NF], BF16, tag="wt")
        nc.scalar.dma_start(wt[:], wt_d)
        thr = const.tile([P, 2 * CW3], F32, tag="thr")
        nc.scalar.dma_start(thr[:], thr_d)
        sfc = const.tile([P, 9 * NOUT], BF16, tag="sfc")
        nc.scalar.dma_start(sfc[:], sfc_d)
        fcb = const.tile([NOUT, 1], F32, tag="fcb")
        nc.scalar.dma_start(fcb[:], fcb_d)

        imh = imp.tile([KH, NK * P], BF16, tag="imh")
        iml = imp.tile([KL, NK * P], BF16, tag="iml")
        for bt in range(BT):
            s = bt * HO * P
            e = (bt + 1) * HO * P
            nc.sync.dma_start(imh[:, s:e], imh_d[:, s:e])
            nc.sync.dma_start(iml[:, s:e], iml_d[:, s:e])

        tT = [ttp.tile([P, B], BF16, tag=f"tT{j}", name=f"tT{j}") for j in range(9)]
        # persistent t' staging tiles, pad columns zeroed once
        tst = [ttp.tile([P, 3 * P], BF16, tag=f"ts{g}", name=f"ts{g}")
               for g in range(12)]
        for g in range(12):
            nc.vector.memset(tst[g][:, CW3:3 * P], 0.0)

        def conv_tile(bt, zp, fc_hook=None):
            zqs = {}
            for h in range(HO):
                k = bt * HO + h
                z = zp.tile([P, NF], F32, tag="z", name="z")
                for n0, n1 in ((0, 512), (512, 1024), (1024, NF)):
                    nc.tensor.matmul(z[:, n0:n1],
                                     lhsT=imh[:, k * P:(k + 1) * P],
                                     rhs=wt[:, n0:n1],
                                     start=True, stop=False)
                    nc.tensor.matmul(z[:, n0:n1],
                                     lhsT=iml[:, k * P:(k + 1) * P],
                                     rhs=wt[0:KL, n0:n1],
                                     start=False, stop=True)
                u = yp.tile([P, CW3], F32, tag="u", name="u")
                nc.vector.reduce_max(
                    u[:], z[:].rearrange("p (ww cw) -> p cw ww", ww=4),
                    axis=mybir.AxisListType.X)
                zqs[h % 4] = u
                if h % 4 == 3:
                    h3 = h // 4
                    m1 = yp.tile([P, CW3], F32, tag="m1", name="m1")
                    nc.vector.tensor_max(m1[:], zqs[0][:], zqs[1][:])
                    m2 = yp.tile([P, CW3], F32, tag="m2", name="m2")
                    nc.vector.tensor_max(m2[:], zqs[2][:], zqs[3][:])
                    m = gp.tile([P, CW3], F32, tag="m", name="m")
                    nc.vector.tensor_max(m[:], m1[:], m2[:])

                    gh = gp.tile([P, CW3], BF16, tag="gh", name="gh")
                    nc.vector.tensor_tensor(gh[:], m[:], thr[:, 0:CW3],
                                            ALU.is_gt)
                    gl = gp.tile([P, CW3], BF16, tag="gl", name="gl")
                    nc.vector.tensor_tensor(gl[:], m[:], thr[:, CW3:2 * CW3],
                                            ALU.is_gt)
                    t_ = tst[bt * 3 + h3]
                    nc.vector.tensor_add(t_[:, 0:CW3], gh[:], gl[:])
                    for ch in range(3):
                        eng = nc.scalar if ch == 1 else nc.sync
                        eng.dma_start_transpose(
                            tT[h3 * 3 + ch][:, bt * P:(bt + 1) * P],
                            t_[:, ch * P:(ch + 1) * P])
                    if fc_hook is not None:
                        fc_hook(h3)

        with tc.tile_pool(name="zp", bufs=2, space="PSUM") as zp:
            for bt in range(BT - 1):
                conv_tile(bt, zp)

            with tc.tile_pool(name="fcp", bufs=1, space="PSUM") as fcp:
                acc = fcp.tile([NOUT, B], F32, tag="acc")

                def fc_hook(h3):
                    for j in (3 * h3, 3 * h3 + 1, 3 * h3 + 2):
                        kj = 32 if j % 3 == 2 else 128
                        nc.tensor.matmul(acc[:, :],
                                         lhsT=sfc[0:kj,
                                                  j * NOUT:(j + 1) * NOUT],
                                         rhs=tT[j][0:kj, :],
                                         start=(j == 0), stop=(j == 8))

                conv_tile(BT - 1, zp, fc_hook=fc_hook)

                ob = const.tile([NOUT, B], F32, tag="ob")
                nc.scalar.activation(ob[:, :], acc[:],
                                     mybir.ActivationFunctionType.Identity,
                                     bias=fcb[0:NOUT, 0:1], scale=1.0)
                nc.sync.dma_start(out_d[:, :], ob[:])

    nc.compile()
    return nc


_NC_CACHE = None
LAST_RESULTS = None


def kernel(x, conv_w, conv_b, bn_gamma, bn_beta, bn_mean, bn_var, fc_w, fc_b):
    global _NC_CACHE, LAST_RESULTS
    x = np.asarray(x, np.float32).reshape(BFULL, H * W)
    wt, thr, sfc, fcb = _host_prep(
        np.asarray(conv_w, np.float64), np.asarray(conv_b, np.float64),
        np.asarray(bn_gamma, np.float64), np.asarray(bn_beta, np.float64),
        np.asarray(bn_mean, np.float64), np.asarray(bn_var, np.float64),
        np.asarray(fc_w, np.float32), np.asarray(fc_b, np.float64))

    if _NC_CACHE is None:
        _NC_CACHE = _build()
    nc = _NC_CACHE

    in_maps = []
    for i in range(NCORES):
        imh, iml = _host_im2col(x[i * B:(i + 1) * B])
        in_maps.append(dict(imh=imh, iml=iml, wt=wt, thr=thr, sfc=sfc, fcb=fcb))
    trace = _os.environ.get("KTRACE", "0") == "1"
    res = run_bass_kernel_spmd(nc, in_maps, core_ids=list(range(NCORES)),
                               trace=trace)
    LAST_RESULTS = res
    out = np.concatenate(
        [np.ascontiguousarray(res.results[i]["out"].T) for i in range(NCORES)],
        axis=0)
    return out.astype(np.float32)


# revision 28
# speedup vs baseline: 1.0147x; 1.0147x over previous
"""Trainium2 Bass kernel for the binarized CNN:
conv3x3(sign weights) -> BN -> ternary hardtanh -> maxpool4 -> linear(sign weights)

Strategy (pure data parallel over batch, 8 cores x 512 samples):
  - Conv as K~116 matmuls with EXACT bf16 operands: x is split hi/lo into two
    bf16 planes on the host (products bf16*{-1,0,1} and bf16 bias rows are
    exact; fp32 PSUM accumulation), so the conv matches the reference within
    the certified ternary-threshold margins while streaming at 2 (not fp32's
    4) PE cycles per output column. Per (batch-tile, row): 6 matmuls
    (2 passes x 3 PSUM banks). Host also builds the full im2col matrix, so
    no on-device transposes or SBUF gather DMAs are needed.
  - BN + ternary thresholds fold into per-channel fp32 thresholds: the conv
    emits Z = sign-conv(x) - tau_mid[c] via two bf16 bias rows; ternary
    t' = (Z>dH[c]) + (Z>dL[c]) in {0,1,2} via tensor-tensor compares
    against replicated threshold tiles; the -1 shift folds into fc bias.
  - maxpool commutes with the monotone BN+ternary: w-pool runs as a
    contiguous-window reduce_max straight from PSUM (interleaved weight
    column order, n = c*36 + w), h-pool as tensor_max over the 4 row tiles.
  - FC: DMA-transpose t' tiles to [feature, batch], 9 accumulating bf16
    matmuls; output written [10, 512], un-transposed on host.
  NOTE: an int16-quantized pooling variant (ScalarE saturating-cast
  evacuation + 2x-rate int16 DVE pool) was ~same speed but showed rare
  nondeterministic corruption on hardware; this fp32 pool path is stable.
"""

import numpy as np
from contextlib import ExitStack

import concourse.bass as bass
import concourse.tile as tile
from concourse import bacc, mybir
from concourse.bass_utils import run_bass_kernel_spmd

import os as _os

F32 = mybir.dt.float32
BF16 = mybir.dt.bfloat16
I16 = mybir.dt.int16
ALU = mybir.AluOpType

# Walrus LDWEIGHTS dedup: crashes codegen (visitInstLdweights) on this
# kernel, so off by default; kept behind an env flag for experiments.
if _os.environ.get("KLDW", "0") == "1":
    from concourse import bass_utils as _bu
    if not getattr(_bu, "_ldw_cmd_patched", False):
        _orig_run_command = _bu.run_command

        def _run_command(cmd, *a, **k):
            cmd = [x if x != "--enable-ldw-opt=false" else "--enable-ldw-opt=true"
                   for x in cmd]
            return _orig_run_command(cmd, *a, **k)

        _bu.run_command = _run_command
        _bu._ldw_cmd_patched = True

NCORES = 8
BFULL = 4096
B = BFULL // NCORES          # 512 per core
P = 128
BT = B // P                  # 4 batch tiles
H, W = 14, 38
HO, WO = 12, 36
C = 32
KH = 116                     # 114 patch rows + 2 bias rows (hi plane)
KL = 114                     # lo plane: patch rows only
NF = C * WO                  # 1152 conv outputs per (b, h)
CW3 = C * (WO // 4)          # 288 after w-pool
EPS = 1e-5
NOUT = 10
NK = BT * HO                 # 48 conv tiles
QS = 16384.0                 # int16 quant scale 2^14


def _host_prep(conv_w, conv_b, bn_gamma, bn_beta, bn_mean, bn_var, fc_w, fc_b):
    import ml_dtypes
    f64 = np.float64
    inv = bn_gamma.astype(f64) / np.sqrt(bn_var.astype(f64) + EPS)
    tauH = (0.5 - bn_beta) / inv + bn_mean - conv_b      # y>0.5  <=> z>tauH
    tauL = (-0.5 - bn_beta) / inv + bn_mean - conv_b
    tmid = 0.5 * (tauH + tauL)
    bh = (-tmid).astype(np.float32).astype(ml_dtypes.bfloat16)
    bl = ((-tmid) - bh.astype(f64)).astype(np.float32).astype(ml_dtypes.bfloat16)
    dH = (tauH - tmid).astype(np.float32)
    dL = (tauL - tmid).astype(np.float32)

    sw = np.sign(conv_w[:, 0]).astype(np.float32)        # [32, 3, 3]
    wt = np.zeros((KH, NF), np.float32)
    for c in range(C):
        for w in range(WO):
            n = c * WO + w                               # interleaved order
            for i in range(3):
                for j in range(3):
                    wt[i * W + w + j, n] = sw[c, i, j]
            wt[114, n] = bh[c]
            wt[115, n] = bl[c]

    thr = np.zeros((P, 2 * CW3), np.float32)
    for c in range(C):
        for w3 in range(9):
            thr[:, c * 9 + w3] = dH[c]
            thr[:, CW3 + c * 9 + w3] = dL[c]

    sf = np.sign(fc_w).astype(np.float32)                # [10, 864]
    sfc = np.zeros((P, 9 * NOUT), np.float32)
    for jj in range(9):
        h3, ch = jj // 3, jj % 3
        kj = 32 if ch == 2 else 128
        for r in range(kj):
            rg = ch * 128 + r                            # index into (c, w3)
            c, w3 = rg // 9, rg % 9
            f = c * 27 + h3 * 9 + w3                     # reference flatten order
            sfc[r, jj * NOUT:(jj + 1) * NOUT] = sf[:, f]

    fcb = (fc_b.astype(f64) - sf.astype(f64).sum(axis=1)).astype(np.float32)
    return (wt.astype(ml_dtypes.bfloat16), thr,
            sfc.astype(ml_dtypes.bfloat16), fcb.reshape(NOUT, 1))


def _host_im2col(xc):
    """xc [512, 532] f32 -> (imh [116, 6144] bf16, iml [114, 6144] bf16),
    columns ordered (bt, h, b)."""
    import ml_dtypes
    xh = xc.astype(ml_dtypes.bfloat16)
    xl = (xc - xh.astype(np.float32)).astype(ml_dtypes.bfloat16)

    def cols(a):
        win = np.lib.stride_tricks.sliding_window_view(a, 114, axis=1)[:, ::W]
        return win.reshape(BT, P, HO, 114).transpose(3, 0, 2, 1).reshape(114, -1)

    imh = np.empty((KH, NK * P), ml_dtypes.bfloat16)
    imh[:114] = cols(xh)
    imh[114:] = 1.0
    iml = np.ascontiguousarray(cols(xl))
    return imh, iml


def _build():
    nc = bacc.Bacc("TRN2", target_bir_lowering=False, debug=False,
                   num_devices=NCORES)
    imh_d = nc.dram_tensor("imh", [KH, NK * P], BF16, kind="ExternalInput").ap()
    iml_d = nc.dram_tensor("iml", [KL, NK * P], BF16, kind="ExternalInput").ap()
    wt_d = nc.dram_tensor("wt", [KH, NF], BF16, kind="ExternalInput").ap()
    thr_d = nc.dram_tensor("thr", [P, 2 * CW3], F32, kind="ExternalInput").ap()
    sfc_d = nc.dram_tensor("sfc", [P, 9 * NOUT], BF16, kind="ExternalInput").ap()
    fcb_d = nc.dram_tensor("fcb", [NOUT, 1], F32, kind="ExternalInput").ap()
    out_d = nc.dram_tensor("out", [NOUT, B], F32, kind="ExternalOutput").ap()

    with tile.TileContext(nc) as tc, ExitStack() as ctx:
        const = ctx.enter_context(tc.tile_pool(name="const", bufs=1))
        imp = ctx.enter_context(tc.tile_pool(name="imp", bufs=1))
        zqp = ctx.enter_context(tc.tile_pool(name="zq", bufs=7))
        yp = ctx.enter_context(tc.tile_pool(name="y", bufs=6))
        gp = ctx.enter_context(tc.tile_pool(name="g", bufs=6))
        ttp = ctx.enter_context(tc.tile_pool(name="tt", bufs=1))

        wt = const.tile([KH, NF], BF16, tag="wt")
        nc.scalar.dma_start(wt[:], wt_d)
        thr = const.tile([P, 2 * CW3], F32, tag="thr")
        nc.scalar.dma_start(thr[:], thr_d)
        sfc = const.tile([P, 9 * NOUT], BF16, tag="sfc")
        nc.scalar.dma_start(sfc[:], sfc_d)
        fcb = const.tile([NOUT, 1], F32, tag="fcb")
        nc.scalar.dma_start(fcb[:], fcb_d)

        imh = imp.tile([KH, NK * P], BF16, tag="imh")
        iml = imp.tile([KL, NK * P], BF16, tag="iml")
        G0 = 4 * P                   # first h-group of bt 0
        nc.sync.dma_start(imh[:, 0:G0], imh_d[:, 0:G0])
        nc.sync.dma_start(iml[:, 0:G0], iml_d[:, 0:G0])
        nc.sync.dma_start(imh[:, G0:HO * P], imh_d[:, G0:HO * P])
        nc.sync.dma_start(iml[:, G0:HO * P], iml_d[:, G0:HO * P])
        for bt in range(1, BT):
            s = bt * HO * P
            e = (bt + 1) * HO * P
            nc.sync.dma_start(imh[:, s:e], imh_d[:, s:e])
            nc.sync.dma_start(iml[:, s:e], iml_d[:, s:e])

        tT = [ttp.tile([P, B], BF16, tag=f"tT{j}", name=f"tT{j}") for j in range(9)]
        # persistent t' staging tiles, pad columns zeroed once
        tst = [ttp.tile([P, 3 * P], BF16, tag=f"ts{g}", name=f"ts{g}")
               for g in range(12)]
        for g in range(12):
            nc.vector.memset(tst[g][:, CW3:3 * P], 0.0)

        def conv_tile(bt, zp, fc_hook=None):
            zqs = {}
            for h in range(HO):
                k = bt * HO + h
                z = zp.tile([P, NF], F32, tag="z", name="z")
                for n0, n1 in ((0, 512), (512, 1024), (1024, NF)):
                    nc.tensor.matmul(z[:, n0:n1],
                                     lhsT=imh[:, k * P:(k + 1) * P],
                                     rhs=wt[:, n0:n1],
                                     start=True, stop=False)
                    nc.tensor.matmul(z[:, n0:n1],
                                     lhsT=iml[:, k * P:(k + 1) * P],
                                     rhs=wt[0:KL, n0:n1],
                                     start=False, stop=True)
                u = yp.tile([P, CW3], F32, tag="u", name="u")
                nc.vector.reduce_max(
                    u[:], z[:].rearrange("p (cw ww) -> p cw ww", ww=4),
                    axis=mybir.AxisListType.X)
                zqs[h % 4] = u
                if h % 4 == 3:
                    # h-pool + ternary on GpSimd (all-float, SBUF-only),
                    # keeping DVE a pure reduce_max pipeline
                    h3 = h // 4
                    m1 = yp.tile([P, CW3], F32, tag="m1", name="m1")
                    nc.vector.tensor_max(m1[:], zqs[0][:], zqs[1][:])
                    m2 = yp.tile([P, CW3], F32, tag="m2", name="m2")
                    nc.vector.tensor_max(m2[:], zqs[2][:], zqs[3][:])
                    m = gp.tile([P, CW3], F32, tag="m", name="m")
                    nc.vector.tensor_max(m[:], m1[:], m2[:])

                    gh = gp.tile([P, CW3], BF16, tag="gh", name="gh")
                    nc.vector.tensor_tensor(gh[:], m[:], thr[:, 0:CW3],
                                            ALU.is_gt)
                    gl = gp.tile([P, CW3], BF16, tag="gl", name="gl")
                    nc.vector.tensor_tensor(gl[:], m[:], thr[:, CW3:2 * CW3],
                                            ALU.is_gt)
                    t_ = tst[bt * 3 + h3]
                    nc.vector.tensor_add(t_[:, 0:CW3], gh[:], gl[:])
                    for ch in range(3):
                        eng = nc.scalar if ch == 1 else nc.sync
                        eng.dma_start_transpose(
                            tT[h3 * 3 + ch][:, bt * P:(bt + 1) * P],
                            t_[:, ch * P:(ch + 1) * P])
                    if fc_hook is not None:
                        fc_hook(h3)

        with tc.tile_pool(name="zp", bufs=2, space="PSUM") as zp:
            for bt in range(BT - 1):
                conv_tile(bt, zp)

            with tc.tile_pool(name="fcp", bufs=1, space="PSUM") as fcp:
                acc = fcp.tile([NOUT, B], F32, tag="acc")

                def fc_hook(h3):
                    for j in (3 * h3, 3 * h3 + 1, 3 * h3 + 2):
                        kj = 32 if j % 3 == 2 else 128
                        nc.tensor.matmul(acc[:, :],
                                         lhsT=sfc[0:kj,
                                                  j * NOUT:(j + 1) * NOUT],
                                         rhs=tT[j][0:kj, :],
                                         start=(j == 0), stop=(j == 8))

                conv_tile(BT - 1, zp, fc_hook=fc_hook)

                ob = const.tile([NOUT, B], F32, tag="ob")
                nc.scalar.activation(ob[:, :], acc[:],
                                     mybir.ActivationFunctionType.Identity,
                                     bias=fcb[0:NOUT, 0:1], scale=1.0)
                nc.sync.dma_start(out_d[:, :], ob[:])

    nc.compile()
    return nc


_NC_CACHE = None
LAST_RESULTS = None


def kernel(x, conv_w, conv_b, bn_gamma, bn_beta, bn_mean, bn_var, fc_w, fc_b):
    global _NC_CACHE, LAST_RESULTS
    x = np.asarray(x, np.float32).reshape(BFULL, H * W)
    wt, thr, sfc, fcb = _host_prep(
        np.asarray(conv_w, np.float64), np.asarray(conv_b, np.float64),
        np.asarray(bn_gamma, np.float64), np.asarray(bn_beta, np.float64),
        np.asarray(bn_mean, np.float64), np.asarray(bn_var, np.float64),
        np.asarray(fc_w, np.float32), np.asarray(fc_b, np.float64))

    if _NC_CACHE is None:
        _NC_CACHE = _build()
    nc = _NC_CACHE

    in_maps = []
    for i in range(NCORES):
        imh, iml = _host_im2col(x[i * B:(i + 1) * B])
        in_maps.append(dict(imh=imh, iml=iml, wt=wt, thr=thr, sfc=sfc, fcb=fcb))
    trace = _os.environ.get("KTRACE", "0") == "1"
    res = run_bass_kernel_spmd(nc, in_maps, core_ids=list(range(NCORES)),
                               trace=trace)
    LAST_RESULTS = res
    out = np.concatenate(
        [np.ascontiguousarray(res.results[i]["out"].T) for i in range(NCORES)],
        axis=0)
    return out.astype(np.float32)


# revision 35
# speedup vs baseline: 1.2025x; 1.1851x over previous
"""Trainium2 Bass kernel for the binarized CNN:
conv3x3(sign weights) -> BN -> ternary hardtanh -> maxpool4 -> linear(sign weights)

Strategy (pure data parallel over batch, 8 cores x 512 samples):
  - Conv as K~116 matmuls with EXACT bf16 operands: x is split hi/lo into two
    bf16 planes on the host (products bf16*{-1,0,1} and bf16 bias rows are
    exact; fp32 PSUM accumulation), so the conv matches the reference within
    the certified ternary-threshold margins while streaming at 2 (not fp32's
    4) PE cycles per output column. Per (batch-tile, row): 6 matmuls
    (2 passes x 3 PSUM banks). Host also builds the full im2col matrix, so
    no on-device transposes or SBUF gather DMAs are needed.
  - BN + ternary thresholds fold into per-channel fp32 thresholds: the conv
    emits Z = sign-conv(x) - tau_mid[c] via two bf16 bias rows; ternary
    t' = (Z>dH[c]) + (Z>dL[c]) in {0,1,2} via tensor-tensor compares
    against replicated threshold tiles; the -1 shift folds into fc bias.
  - maxpool commutes with the monotone BN+ternary: w-pool runs as a
    contiguous-window reduce_max straight from PSUM (interleaved weight
    column order, n = c*36 + w), h-pool as tensor_max over the 4 row tiles.
  - FC: DMA-transpose t' tiles to [feature, batch], 9 accumulating bf16
    matmuls; output written [10, 512], un-transposed on host.
  NOTE: an int16-quantized pooling variant (ScalarE saturating-cast
  evacuation + 2x-rate int16 DVE pool) was ~same speed but showed rare
  nondeterministic corruption on hardware; this fp32 pool path is stable.
"""

import numpy as np
from contextlib import ExitStack

import concourse.bass as bass
import concourse.tile as tile
from concourse import bacc, mybir
from concourse.bass_utils import run_bass_kernel_spmd

import os as _os

F32 = mybir.dt.float32
BF16 = mybir.dt.bfloat16
I16 = mybir.dt.int16
ALU = mybir.AluOpType

# Walrus LDWEIGHTS dedup: crashes codegen (visitInstLdweights) on this
# kernel, so off by default; kept behind an env flag for experiments.
if _os.environ.get("KLDW", "0") == "1":
    from concourse import bass_utils as _bu
    if not getattr(_bu, "_ldw_cmd_patched", False):
        _orig_run_command = _bu.run_command

        def _run_command(cmd, *a, **k):
            cmd = [x if x != "--enable-ldw-opt=false" else "--enable-ldw-opt=true"
                   for x in cmd]
            return _orig_run_command(cmd, *a, **k)

        _bu.run_command = _run_command
        _bu._ldw_cmd_patched = True

NCORES = 8
BFULL = 4096
B = BFULL // NCORES          # 512 per core
P = 128
BT = B // P                  # 4 batch tiles
H, W = 14, 38
HO, WO = 12, 36
C = 32
KH = 116                     # 114 patch rows + 2 bias rows (hi plane)
KL = 114                     # lo plane: patch rows only
NF = C * WO                  # 1152 conv outputs per (b, h)
CW3 = C * (WO // 4)          # 288 after w-pool
EPS = 1e-5
NOUT = 10
NK = BT * HO                 # 48 conv tiles
QS = 16384.0                 # int16 quant scale 2^14


def _host_prep(conv_w, conv_b, bn_gamma, bn_beta, bn_mean, bn_var, fc_w, fc_b):
    import ml_dtypes
    f64 = np.float64
    inv = bn_gamma.astype(f64) / np.sqrt(bn_var.astype(f64) + EPS)
    tauH = (0.5 - bn_beta) / inv + bn_mean - conv_b      # y>0.5  <=> z>tauH
    tauL = (-0.5 - bn_beta) / inv + bn_mean - conv_b
    tmid = 0.5 * (tauH + tauL)
    bh = (-tmid).astype(np.float32).astype(ml_dtypes.bfloat16)
    bl = ((-tmid) - bh.astype(f64)).astype(np.float32).astype(ml_dtypes.bfloat16)
    dH = (tauH - tmid).astype(np.float32)
    dL = (tauL - tmid).astype(np.float32)

    sw = np.sign(conv_w[:, 0]).astype(np.float32)        # [32, 3, 3]
    wt = np.zeros((KH, NF), np.float32)
    for c in range(C):
        for w in range(WO):
            n = c * WO + w                               # interleaved order
            for i in range(3):
                for j in range(3):
                    wt[i * W + w + j, n] = sw[c, i, j]
            wt[114, n] = bh[c]
            wt[115, n] = bl[c]

    thr = np.zeros((P, 2 * CW3), np.float32)
    for c in range(C):
        for w3 in range(9):
            thr[:, c * 9 + w3] = dH[c]
            thr[:, CW3 + c * 9 + w3] = dL[c]

    sf = np.sign(fc_w).astype(np.float32)                # [10, 864]
    sfc = np.zeros((P, 9 * NOUT), np.float32)
    for jj in range(9):
        h3, ch = jj // 3, jj % 3
        kj = 32 if ch == 2 else 128
        for r in range(kj):
            rg = ch * 128 + r                            # index into (c, w3)
            c, w3 = rg // 9, rg % 9
            f = c * 27 + h3 * 9 + w3                     # reference flatten order
            sfc[r, jj * NOUT:(jj + 1) * NOUT] = sf[:, f]

    fcb = (fc_b.astype(f64) - sf.astype(f64).sum(axis=1)).astype(np.float32)
    return (wt.astype(ml_dtypes.bfloat16), thr,
            sfc.astype(ml_dtypes.bfloat16), fcb.reshape(NOUT, 1))


def _host_im2col(xc):
    """xc [512, 532] f32 -> (imh [116, 6144] bf16, iml [114, 6144] bf16),
    columns ordered (bt, h, b)."""
    import ml_dtypes
    xh = xc.astype(ml_dtypes.bfloat16)
    xl = (xc - xh.astype(np.float32)).astype(ml_dtypes.bfloat16)

    def cols(a):
        win = np.lib.stride_tricks.sliding_window_view(a, 114, axis=1)[:, ::W]
        return win.reshape(BT, P, HO, 114).transpose(3, 0, 2, 1).reshape(114, -1)

    imh = np.empty((KH, NK * P), ml_dtypes.bfloat16)
    imh[:114] = cols(xh)
    imh[114:] = 1.0
    iml = np.ascontiguousarray(cols(xl))
    return imh, iml


def _build():
    nc = bacc.Bacc("TRN2", target_bir_lowering=False, debug=False,
                   num_devices=NCORES)
    imh_d = nc.dram_tensor("imh", [KH, NK * P], BF16, kind="ExternalInput").ap()
    iml_d = nc.dram_tensor("iml", [KL, NK * P], BF16, kind="ExternalInput").ap()
    wt_d = nc.dram_tensor("wt", [KH, NF], BF16, kind="ExternalInput").ap()
    thr_d = nc.dram_tensor("thr", [P, 2 * CW3], F32, kind="ExternalInput").ap()
    sfc_d = nc.dram_tensor("sfc", [P, 9 * NOUT], BF16, kind="ExternalInput").ap()
    fcb_d = nc.dram_tensor("fcb", [NOUT, 1], F32, kind="ExternalInput").ap()
    out_d = nc.dram_tensor("out", [NOUT, B], F32, kind="ExternalOutput").ap()

    with tile.TileContext(nc) as tc, ExitStack() as ctx:
        const = ctx.enter_context(tc.tile_pool(name="const", bufs=1))
        imp = ctx.enter_context(tc.tile_pool(name="imp", bufs=1))
        zqp = ctx.enter_context(tc.tile_pool(name="zq", bufs=7))
        yp = ctx.enter_context(tc.tile_pool(name="y", bufs=6))
        gp = ctx.enter_context(tc.tile_pool(name="g", bufs=6))
        ttp = ctx.enter_context(tc.tile_pool(name="tt", bufs=1))

        wt = const.tile([KH, NF], BF16, tag="wt")
        nc.scalar.dma_start(wt[:], wt_d)
        thr = const.tile([P, 2 * CW3], F32, tag="thr")
        nc.scalar.dma_start(thr[:], thr_d)
        sfc = const.tile([P, 9 * NOUT], BF16, tag="sfc")
        nc.scalar.dma_start(sfc[:], sfc_d)
        fcb = const.tile([NOUT, 1], F32, tag="fcb")
        nc.scalar.dma_start(fcb[:], fcb_d)

        imh = imp.tile([KH, NK * P], BF16, tag="imh")
        iml = imp.tile([KL, NK * P], BF16, tag="iml")
        G0 = 4 * P                   # first h-group of bt 0
        nc.sync.dma_start(imh[:, 0:G0], imh_d[:, 0:G0])
        nc.sync.dma_start(iml[:, 0:G0], iml_d[:, 0:G0])
        nc.sync.dma_start(imh[:, G0:HO * P], imh_d[:, G0:HO * P])
        nc.sync.dma_start(iml[:, G0:HO * P], iml_d[:, G0:HO * P])
        for bt in range(1, BT):
            s = bt * HO * P
            e = (bt + 1) * HO * P
            nc.sync.dma_start(imh[:, s:e], imh_d[:, s:e])
            nc.sync.dma_start(iml[:, s:e], iml_d[:, s:e])

        tT = [ttp.tile([P, B], BF16, tag=f"tT{j}", name=f"tT{j}") for j in range(9)]
        # persistent t' staging tiles, pad columns zeroed once
        tst = [ttp.tile([P, 3 * P], BF16, tag=f"ts{g}", name=f"ts{g}")
               for g in range(12)]
        for g in range(12):
            nc.vector.memset(tst[g][:, CW3:3 * P], 0.0)

        def conv_tile(bt, zp, fc_hook=None):
            zqs = {}
            for h in range(HO):
                k = bt * HO + h
                z = zp.tile([P, NF], F32, tag="z", name="z")
                for n0, n1 in ((0, 512), (512, 1024), (1024, NF)):
                    nc.tensor.matmul(z[:, n0:n1],
                                     lhsT=imh[:, k * P:(k + 1) * P],
                                     rhs=wt[:, n0:n1],
                                     start=True, stop=False)
                    nc.tensor.matmul(z[:, n0:n1],
                                     lhsT=iml[:, k * P:(k + 1) * P],
                                     rhs=wt[0:KL, n0:n1],
                                     start=False, stop=True)
                u = yp.tile([P, CW3], F32, tag="u", name="u")
                nc.vector.reduce_max(
                    u[:], z[:].rearrange("p (cw ww) -> p cw ww", ww=4),
                    axis=mybir.AxisListType.X)
                zqs[h % 4] = u
                if h % 4 == 3:
                    h3 = h // 4
                    m1 = yp.tile([P, CW3], F32, tag="m1", name="m1")
                    nc.vector.tensor_max(m1[:], zqs[0][:], zqs[1][:])
                    m2 = yp.tile([P, CW3], F32, tag="m2", name="m2")
                    nc.vector.tensor_max(m2[:], zqs[2][:], zqs[3][:])
                    m = gp.tile([P, CW3], F32, tag="m", name="m")
                    nc.vector.tensor_max(m[:], m1[:], m2[:])

                    gh = gp.tile([P, CW3], BF16, tag="gh", name="gh")
                    nc.vector.tensor_tensor(gh[:], m[:], thr[:, 0:CW3],
                                            ALU.is_gt)
                    gl = gp.tile([P, CW3], BF16, tag="gl", name="gl")
                    nc.vector.tensor_tensor(gl[:], m[:], thr[:, CW3:2 * CW3],
                                            ALU.is_gt)
                    t_ = tst[bt * 3 + h3]
                    nc.vector.tensor_add(t_[:, 0:CW3], gh[:], gl[:])
                    for ch in range(3):
                        eng = nc.scalar if ch == 1 else nc.sync
                        eng.dma_start_transpose(
                            tT[h3 * 3 + ch][:, bt * P:(bt + 1) * P],
                            t_[:, ch * P:(ch + 1) * P])
                    if fc_hook is not None:
                        fc_hook(h3)

        with tc.tile_pool(name="zp", bufs=2, space="PSUM") as zp:
            with tc.tile_pool(name="fcp", bufs=1, space="PSUM") as fcp:
                # two half-batch accumulators in separate banks: the bt0/bt1
                # half of the FC + output runs mid-kernel, shrinking the tail
                acc0 = fcp.tile([NOUT, B // 2], F32, tag="acc0")
                acc1 = fcp.tile([NOUT, B // 2], F32, tag="acc1")
                ob = const.tile([NOUT, B], F32, tag="ob")

                def fc_half(acc, c0):
                    for j in range(9):
                        kj = 32 if j % 3 == 2 else 128
                        nc.tensor.matmul(acc[:, :],
                                         lhsT=sfc[0:kj,
                                                  j * NOUT:(j + 1) * NOUT],
                                         rhs=tT[j][0:kj, c0:c0 + B // 2],
                                         start=(j == 0), stop=(j == 8))
                    nc.scalar.activation(ob[:, c0:c0 + B // 2], acc[:],
                                         mybir.ActivationFunctionType.Identity,
                                         bias=fcb[0:NOUT, 0:1], scale=1.0)
                    nc.sync.dma_start(out_d[:, c0:c0 + B // 2],
                                      ob[:, c0:c0 + B // 2])

                conv_tile(0, zp)
                conv_tile(1, zp)
                conv_tile(2, zp,
                          fc_hook=lambda h3: fc_half(acc0, 0) if h3 == 0
                          else None)
                conv_tile(3, zp)
                fc_half(acc1, B // 2)

    nc.compile()
    return nc


_NC_CACHE = None
LAST_RESULTS = None


def kernel(x, conv_w, conv_b, bn_gamma, bn_beta, bn_mean, bn_var, fc_w, fc_b):
    global _NC_CACHE, LAST_RESULTS
    x = np.asarray(x, np.float32).reshape(BFULL, H * W)
    wt, thr, sfc, fcb = _host_prep(
        np.asarray(conv_w, np.float64), np.asarray(conv_b, np.float64),
        np.asarray(bn_gamma, np.float64), np.asarray(bn_beta, np.float64),
        np.asarray(bn_mean, np.float64), np.asarray(bn_var, np.float64),
        np.asarray(fc_w, np.float32), np.asarray(fc_b, np.float64))

    if _NC_CACHE is None:
        _NC_CACHE = _build()
    nc = _NC_CACHE

    in_maps = []
    for i in range(NCORES):
        imh, iml = _host_im2col(x[i * B:(i + 1) * B])
        in_maps.append(dict(imh=imh, iml=iml, wt=wt, thr=thr, sfc=sfc, fcb=fcb))
    trace = _os.environ.get("KTRACE", "0") == "1"
    res = run_bass_kernel_spmd(nc, in_maps, core_ids=list(range(NCORES)),
                               trace=trace)
    LAST_RESULTS = res
    out = np.concatenate(
        [np.ascontiguousarray(res.results[i]["out"].T) for i in range(NCORES)],
        axis=0)
    return out.astype(np.float32)


# revision 36
# speedup vs baseline: 1.2102x; 1.0064x over previous
"""Trainium2 Bass kernel for the binarized CNN:
conv3x3(sign weights) -> BN -> ternary hardtanh -> maxpool4 -> linear(sign weights)

Strategy (pure data parallel over batch, 8 cores x 512 samples):
  - Conv as K~116 matmuls with EXACT bf16 operands: x is split hi/lo into two
    bf16 planes on the host (products bf16*{-1,0,1} and bf16 bias rows are
    exact; fp32 PSUM accumulation), so the conv matches the reference within
    the certified ternary-threshold margins while streaming at 2 (not fp32's
    4) PE cycles per output column. Per (batch-tile, row): 6 matmuls
    (2 passes x 3 PSUM banks). Host also builds the full im2col matrix, so
    no on-device transposes or SBUF gather DMAs are needed.
  - BN + ternary thresholds fold into per-channel fp32 thresholds: the conv
    emits Z = sign-conv(x) - tau_mid[c] via two bf16 bias rows; ternary
    t' = (Z>dH[c]) + (Z>dL[c]) in {0,1,2} via tensor-tensor compares
    against replicated threshold tiles; the -1 shift folds into fc bias.
  - maxpool commutes with the monotone BN+ternary: w-pool runs as a
    contiguous-window reduce_max straight from PSUM (interleaved weight
    column order, n = c*36 + w), h-pool as tensor_max over the 4 row tiles.
  - FC: DMA-transpose t' tiles to [feature, batch], 9 accumulating bf16
    matmuls; output written [10, 512], un-transposed on host.
  NOTE: an int16-quantized pooling variant (ScalarE saturating-cast
  evacuation + 2x-rate int16 DVE pool) was ~same speed but showed rare
  nondeterministic corruption on hardware; this fp32 pool path is stable.
"""

import numpy as np
from contextlib import ExitStack

import concourse.bass as bass
import concourse.tile as tile
from concourse import bacc, mybir
from concourse.bass_utils import run_bass_kernel_spmd

import os as _os

F32 = mybir.dt.float32
BF16 = mybir.dt.bfloat16
I16 = mybir.dt.int16
ALU = mybir.AluOpType

# Walrus LDWEIGHTS dedup: crashes codegen (visitInstLdweights) on this
# kernel, so off by default; kept behind an env flag for experiments.
if _os.environ.get("KLDW", "0") == "1":
    from concourse import bass_utils as _bu
    if not getattr(_bu, "_ldw_cmd_patched", False):
        _orig_run_command = _bu.run_command

        def _run_command(cmd, *a, **k):
            cmd = [x if x != "--enable-ldw-opt=false" else "--enable-ldw-opt=true"
                   for x in cmd]
            return _orig_run_command(cmd, *a, **k)

        _bu.run_command = _run_command
        _bu._ldw_cmd_patched = True

NCORES = 8
BFULL = 4096
B = BFULL // NCORES          # 512 per core
P = 128
BT = B // P                  # 4 batch tiles
H, W = 14, 38
HO, WO = 12, 36
C = 32
KH = 116                     # 114 patch rows + 2 bias rows (hi plane)
KL = 114                     # lo plane: patch rows only
NF = C * WO                  # 1152 conv outputs per (b, h)
CW3 = C * (WO // 4)          # 288 after w-pool
EPS = 1e-5
NOUT = 10
NK = BT * HO                 # 48 conv tiles
QS = 16384.0                 # int16 quant scale 2^14


def _host_prep(conv_w, conv_b, bn_gamma, bn_beta, bn_mean, bn_var, fc_w, fc_b):
    import ml_dtypes
    f64 = np.float64
    inv = bn_gamma.astype(f64) / np.sqrt(bn_var.astype(f64) + EPS)
    tauH = (0.5 - bn_beta) / inv + bn_mean - conv_b      # y>0.5  <=> z>tauH
    tauL = (-0.5 - bn_beta) / inv + bn_mean - conv_b
    tmid = 0.5 * (tauH + tauL)
    bh = (-tmid).astype(np.float32).astype(ml_dtypes.bfloat16)
    bl = ((-tmid) - bh.astype(f64)).astype(np.float32).astype(ml_dtypes.bfloat16)
    dH = (tauH - tmid).astype(np.float32)
    dL = (tauL - tmid).astype(np.float32)

    sw = np.sign(conv_w[:, 0]).astype(np.float32)        # [32, 3, 3]
    wt = np.zeros((KH, NF), np.float32)
    for c in range(C):
        for w in range(WO):
            n = c * WO + w                               # interleaved order
            for i in range(3):
                for j in range(3):
                    wt[i * W + w + j, n] = sw[c, i, j]
            wt[114, n] = bh[c]
            wt[115, n] = bl[c]

    thr = np.zeros((P, 2 * CW3), np.float32)
    for c in range(C):
        for w3 in range(9):
            thr[:, c * 9 + w3] = dH[c]
            thr[:, CW3 + c * 9 + w3] = dL[c]

    sf = np.sign(fc_w).astype(np.float32)                # [10, 864]
    sfc = np.zeros((P, 9 * NOUT), np.float32)
    for jj in range(9):
        h3, ch = jj // 3, jj % 3
        kj = 32 if ch == 2 else 128
        for r in range(kj):
            rg = ch * 128 + r                            # index into (c, w3)
            c, w3 = rg // 9, rg % 9
            f = c * 27 + h3 * 9 + w3                     # reference flatten order
            sfc[r, jj * NOUT:(jj + 1) * NOUT] = sf[:, f]

    fcb = (fc_b.astype(f64) - sf.astype(f64).sum(axis=1)).astype(np.float32)
    return (wt.astype(ml_dtypes.bfloat16), thr,
            sfc.astype(ml_dtypes.bfloat16), fcb.reshape(NOUT, 1))


def _host_im2col(xc):
    """xc [512, 532] f32 -> (imh [116, 6144] bf16, iml [114, 6144] bf16),
    columns ordered (bt, h, b)."""
    import ml_dtypes
    xh = xc.astype(ml_dtypes.bfloat16)
    xl = (xc - xh.astype(np.float32)).astype(ml_dtypes.bfloat16)

    def cols(a):
        win = np.lib.stride_tricks.sliding_window_view(a, 114, axis=1)[:, ::W]
        return win.reshape(BT, P, HO, 114).transpose(3, 0, 2, 1).reshape(114, -1)

    imh = np.empty((KH, NK * P), ml_dtypes.bfloat16)
    imh[:114] = cols(xh)
    imh[114:] = 1.0
    iml = np.ascontiguousarray(cols(xl))
    return imh, iml


def _build():
    nc = bacc.Bacc("TRN2", target_bir_lowering=False, debug=False,
                   num_devices=NCORES)
    imh_d = nc.dram_tensor("imh", [KH, NK * P], BF16, kind="ExternalInput").ap()
    iml_d = nc.dram_tensor("iml", [KL, NK * P], BF16, kind="ExternalInput").ap()
    wt_d = nc.dram_tensor("wt", [KH, NF], BF16, kind="ExternalInput").ap()
    thr_d = nc.dram_tensor("thr", [P, 2 * CW3], F32, kind="ExternalInput").ap()
    sfc_d = nc.dram_tensor("sfc", [P, 9 * NOUT], BF16, kind="ExternalInput").ap()
    fcb_d = nc.dram_tensor("fcb", [NOUT, 1], F32, kind="ExternalInput").ap()
    out_d = nc.dram_tensor("out", [NOUT, B], F32, kind="ExternalOutput").ap()

    with tile.TileContext(nc) as tc, ExitStack() as ctx:
        const = ctx.enter_context(tc.tile_pool(name="const", bufs=1))
        imp = ctx.enter_context(tc.tile_pool(name="imp", bufs=1))
        zqp = ctx.enter_context(tc.tile_pool(name="zq", bufs=7))
        yp = ctx.enter_context(tc.tile_pool(name="y", bufs=6))
        gp = ctx.enter_context(tc.tile_pool(name="g", bufs=6))
        ttp = ctx.enter_context(tc.tile_pool(name="tt", bufs=1))

        wt = const.tile([KH, NF], BF16, tag="wt")
        nc.scalar.dma_start(wt[:], wt_d)
        thr = const.tile([P, 2 * CW3], F32, tag="thr")
        nc.scalar.dma_start(thr[:], thr_d)
        sfc = const.tile([P, 9 * NOUT], BF16, tag="sfc")
        nc.scalar.dma_start(sfc[:], sfc_d)
        fcb = const.tile([NOUT, 1], F32, tag="fcb")
        nc.scalar.dma_start(fcb[:], fcb_d)

        # per-batch-tile im2col tiles: a conv matmul depends only on its
        # own chunk's DMA, so the first tile starts as soon as chunk 0 lands
        imh_t = [imp.tile([KH, HO * P], BF16, tag=f"imh{bt}", name=f"imh{bt}")
                 for bt in range(BT)]
        iml_t = [imp.tile([KL, HO * P], BF16, tag=f"iml{bt}", name=f"iml{bt}")
                 for bt in range(BT)]
        G0 = 4 * P                   # first h-group of bt 0
        nc.sync.dma_start(imh_t[0][:, 0:G0], imh_d[:, 0:G0])
        nc.sync.dma_start(iml_t[0][:, 0:G0], iml_d[:, 0:G0])
        nc.sync.dma_start(imh_t[0][:, G0:HO * P], imh_d[:, G0:HO * P])
        nc.sync.dma_start(iml_t[0][:, G0:HO * P], iml_d[:, G0:HO * P])
        for bt in range(1, BT):
            s = bt * HO * P
            e = (bt + 1) * HO * P
            nc.sync.dma_start(imh_t[bt][:, :], imh_d[:, s:e])
            nc.sync.dma_start(iml_t[bt][:, :], iml_d[:, s:e])

        tT = [ttp.tile([P, B], BF16, tag=f"tT{j}", name=f"tT{j}") for j in range(9)]
        # persistent t' staging tiles, pad columns zeroed once
        tst = [ttp.tile([P, 3 * P], BF16, tag=f"ts{g}", name=f"ts{g}")
               for g in range(12)]
        for g in range(12):
            nc.vector.memset(tst[g][:, CW3:3 * P], 0.0)

        def conv_tile(bt, zp, fc_hook=None):
            zqs = {}
            for h in range(HO):
                k = bt * HO + h
                z = zp.tile([P, NF], F32, tag="z", name="z")
                for n0, n1 in ((0, 512), (512, 1024), (1024, NF)):
                    nc.tensor.matmul(z[:, n0:n1],
                                     lhsT=imh_t[bt][:, h * P:(h + 1) * P],
                                     rhs=wt[:, n0:n1],
                                     start=True, stop=False)
                    nc.tensor.matmul(z[:, n0:n1],
                                     lhsT=iml_t[bt][:, h * P:(h + 1) * P],
                                     rhs=wt[0:KL, n0:n1],
                                     start=False, stop=True)
                u = yp.tile([P, CW3], F32, tag="u", name="u")
                nc.vector.reduce_max(
                    u[:], z[:].rearrange("p (cw ww) -> p cw ww", ww=4),
                    axis=mybir.AxisListType.X)
                zqs[h % 4] = u
                if h % 4 == 3:
                    h3 = h // 4
                    m1 = yp.tile([P, CW3], F32, tag="m1", name="m1")
                    nc.vector.tensor_max(m1[:], zqs[0][:], zqs[1][:])
                    m2 = yp.tile([P, CW3], F32, tag="m2", name="m2")
                    nc.vector.tensor_max(m2[:], zqs[2][:], zqs[3][:])
                    m = gp.tile([P, CW3], F32, tag="m", name="m")
                    nc.vector.tensor_max(m[:], m1[:], m2[:])

                    gh = gp.tile([P, CW3], BF16, tag="gh", name="gh")
                    nc.vector.tensor_tensor(gh[:], m[:], thr[:, 0:CW3],
                                            ALU.is_gt)
                    gl = gp.tile([P, CW3], BF16, tag="gl", name="gl")
                    nc.vector.tensor_tensor(gl[:], m[:], thr[:, CW3:2 * CW3],
                                            ALU.is_gt)
                    t_ = tst[bt * 3 + h3]
                    nc.vector.tensor_add(t_[:, 0:CW3], gh[:], gl[:])
                    for ch in range(3):
                        eng = nc.scalar if ch == 1 else nc.sync
                        eng.dma_start_transpose(
                            tT[h3 * 3 + ch][:, bt * P:(bt + 1) * P],
                            t_[:, ch * P:(ch + 1) * P])
                    if fc_hook is not None:
                        fc_hook(h3)

        with tc.tile_pool(name="zp", bufs=2, space="PSUM") as zp:
            with tc.tile_pool(name="fcp", bufs=1, space="PSUM") as fcp:
                # two half-batch accumulators in separate banks: the bt0/bt1
                # half of the FC + output runs mid-kernel, shrinking the tail
                acc0 = fcp.tile([NOUT, B // 2], F32, tag="acc0")
                acc1 = fcp.tile([NOUT, B // 2], F32, tag="acc1")
                ob = const.tile([NOUT, B], F32, tag="ob")

                def fc_half(acc, c0):
                    for j in range(9):
                        kj = 32 if j % 3 == 2 else 128
                        nc.tensor.matmul(acc[:, :],
                                         lhsT=sfc[0:kj,
                                                  j * NOUT:(j + 1) * NOUT],
                                         rhs=tT[j][0:kj, c0:c0 + B // 2],
                                         start=(j == 0), stop=(j == 8))
                    nc.scalar.activation(ob[:, c0:c0 + B // 2], acc[:],
                                         mybir.ActivationFunctionType.Identity,
                                         bias=fcb[0:NOUT, 0:1], scale=1.0)
                    nc.sync.dma_start(out_d[:, c0:c0 + B // 2],
                                      ob[:, c0:c0 + B // 2])

                conv_tile(0, zp)
                conv_tile(1, zp)
                conv_tile(2, zp,
                          fc_hook=lambda h3: fc_half(acc0, 0) if h3 == 0
                          else None)
                conv_tile(3, zp)
                fc_half(acc1, B // 2)

    nc.compile()
    return nc


_NC_CACHE = None
LAST_RESULTS = None


def kernel(x, conv_w, conv_b, bn_gamma, bn_beta, bn_mean, bn_var, fc_w, fc_b):
    global _NC_CACHE, LAST_RESULTS
    x = np.asarray(x, np.float32).reshape(BFULL, H * W)
    wt, thr, sfc, fcb = _host_prep(
        np.asarray(conv_w, np.float64), np.asarray(conv_b, np.float64),
        np.asarray(bn_gamma, np.float64), np.asarray(bn_beta, np.float64),
        np.asarray(bn_mean, np.float64), np.asarray(bn_var, np.float64),
        np.asarray(fc_w, np.float32), np.asarray(fc_b, np.float64))

    if _NC_CACHE is None:
        _NC_CACHE = _build()
    nc = _NC_CACHE

    in_maps = []
    for i in range(NCORES):
        imh, iml = _host_im2col(x[i * B:(i + 1) * B])
        in_maps.append(dict(imh=imh, iml=iml, wt=wt, thr=thr, sfc=sfc, fcb=fcb))
    trace = _os.environ.get("KTRACE", "0") == "1"
    res = run_bass_kernel_spmd(nc, in_maps, core_ids=list(range(NCORES)),
                               trace=trace)
    LAST_RESULTS = res
    out = np.concatenate(
        [np.ascontiguousarray(res.results[i]["out"].T) for i in range(NCORES)],
        axis=0)
    return out.astype(np.float32)


# revision 37
# speedup vs baseline: 1.2116x; 1.0011x over previous
"""Trainium2 Bass kernel for the binarized CNN:
conv3x3(sign weights) -> BN -> ternary hardtanh -> maxpool4 -> linear(sign weights)

Strategy (pure data parallel over batch, 8 cores x 512 samples):
  - Conv as K~116 matmuls with EXACT bf16 operands: x is split hi/lo into two
    bf16 planes on the host (products bf16*{-1,0,1} and bf16 bias rows are
    exact; fp32 PSUM accumulation), so the conv matches the reference within
    the certified ternary-threshold margins while streaming at 2 (not fp32's
    4) PE cycles per output column. Per (batch-tile, row): 6 matmuls
    (2 passes x 3 PSUM banks). Host also builds the full im2col matrix, so
    no on-device transposes or SBUF gather DMAs are needed.
  - BN + ternary thresholds fold into per-channel fp32 thresholds: the conv
    emits Z = sign-conv(x) - tau_mid[c] via two bf16 bias rows; ternary
    t' = (Z>dH[c]) + (Z>dL[c]) in {0,1,2} via tensor-tensor compares
    against replicated threshold tiles; the -1 shift folds into fc bias.
  - maxpool commutes with the monotone BN+ternary: w-pool runs as a
    contiguous-window reduce_max straight from PSUM (interleaved weight
    column order, n = c*36 + w), h-pool as tensor_max over the 4 row tiles.
  - FC: DMA-transpose t' tiles to [feature, batch], 9 accumulating bf16
    matmuls; output written [10, 512], un-transposed on host.
  NOTE: an int16-quantized pooling variant (ScalarE saturating-cast
  evacuation + 2x-rate int16 DVE pool) was ~same speed but showed rare
  nondeterministic corruption on hardware; this fp32 pool path is stable.
"""

import numpy as np
from contextlib import ExitStack

import concourse.bass as bass
import concourse.tile as tile
from concourse import bacc, mybir
from concourse.bass_utils import run_bass_kernel_spmd

import os as _os

F32 = mybir.dt.float32
BF16 = mybir.dt.bfloat16
I16 = mybir.dt.int16
ALU = mybir.AluOpType

# Walrus LDWEIGHTS dedup: crashes codegen (visitInstLdweights) on this
# kernel, so off by default; kept behind an env flag for experiments.
if _os.environ.get("KLDW", "0") == "1":
    from concourse import bass_utils as _bu
    if not getattr(_bu, "_ldw_cmd_patched", False):
        _orig_run_command = _bu.run_command

        def _run_command(cmd, *a, **k):
            cmd = [x if x != "--enable-ldw-opt=false" else "--enable-ldw-opt=true"
                   for x in cmd]
            return _orig_run_command(cmd, *a, **k)

        _bu.run_command = _run_command
        _bu._ldw_cmd_patched = True

NCORES = 8
BFULL = 4096
B = BFULL // NCORES          # 512 per core
P = 128
BT = B // P                  # 4 batch tiles
H, W = 14, 38
HO, WO = 12, 36
C = 32
KH = 116                     # 114 patch rows + 2 bias rows (hi plane)
KL = 114                     # lo plane: patch rows only
NF = C * WO                  # 1152 conv outputs per (b, h)
CW3 = C * (WO // 4)          # 288 after w-pool
EPS = 1e-5
NOUT = 10
NK = BT * HO                 # 48 conv tiles
QS = 16384.0                 # int16 quant scale 2^14


def _host_prep(conv_w, conv_b, bn_gamma, bn_beta, bn_mean, bn_var, fc_w, fc_b):
    import ml_dtypes
    f64 = np.float64
    inv = bn_gamma.astype(f64) / np.sqrt(bn_var.astype(f64) + EPS)
    tauH = (0.5 - bn_beta) / inv + bn_mean - conv_b      # y>0.5  <=> z>tauH
    tauL = (-0.5 - bn_beta) / inv + bn_mean - conv_b
    tmid = 0.5 * (tauH + tauL)
    bh = (-tmid).astype(np.float32).astype(ml_dtypes.bfloat16)
    bl = ((-tmid) - bh.astype(f64)).astype(np.float32).astype(ml_dtypes.bfloat16)
    dH = (tauH - tmid).astype(np.float32)
    dL = (tauL - tmid).astype(np.float32)

    sw = np.sign(conv_w[:, 0]).astype(np.float32)        # [32, 3, 3]
    wt = np.zeros((KH, NF), np.float32)
    for c in range(C):
        for w in range(WO):
            n = c * WO + w                               # interleaved order
            for i in range(3):
                for j in range(3):
                    wt[i * W + w + j, n] = sw[c, i, j]
            wt[114, n] = bh[c]
            wt[115, n] = bl[c]

    thr = np.zeros((P, 2 * CW3), np.float32)
    for c in range(C):
        for w3 in range(9):
            thr[:, c * 9 + w3] = dH[c]
            thr[:, CW3 + c * 9 + w3] = dL[c]

    sf = np.sign(fc_w).astype(np.float32)                # [10, 864]
    sfc = np.zeros((P, 9 * NOUT), np.float32)
    for jj in range(9):
        h3, ch = jj // 3, jj % 3
        kj = 32 if ch == 2 else 128
        for r in range(kj):
            rg = ch * 128 + r                            # index into (c, w3)
            c, w3 = rg // 9, rg % 9
            f = c * 27 + h3 * 9 + w3                     # reference flatten order
            sfc[r, jj * NOUT:(jj + 1) * NOUT] = sf[:, f]

    fcb = (fc_b.astype(f64) - sf.astype(f64).sum(axis=1)).astype(np.float32)
    return (wt.astype(ml_dtypes.bfloat16), thr,
            sfc.astype(ml_dtypes.bfloat16), fcb.reshape(NOUT, 1))


def _host_im2col(xc):
    """xc [512, 532] f32 -> (imh [116, 6144] bf16, iml [114, 6144] bf16),
    columns ordered (bt, h, b)."""
    import ml_dtypes
    xh = xc.astype(ml_dtypes.bfloat16)
    xl = (xc - xh.astype(np.float32)).astype(ml_dtypes.bfloat16)

    def cols(a):
        win = np.lib.stride_tricks.sliding_window_view(a, 114, axis=1)[:, ::W]
        return win.reshape(BT, P, HO, 114).transpose(3, 0, 2, 1).reshape(114, -1)

    imh = np.empty((KH, NK * P), ml_dtypes.bfloat16)
    imh[:114] = cols(xh)
    imh[114:] = 1.0
    iml = np.ascontiguousarray(cols(xl))
    return imh, iml


def _build():
    nc = bacc.Bacc("TRN2", target_bir_lowering=False, debug=False,
                   num_devices=NCORES)
    imh_d = nc.dram_tensor("imh", [KH, NK * P], BF16, kind="ExternalInput").ap()
    iml_d = nc.dram_tensor("iml", [KL, NK * P], BF16, kind="ExternalInput").ap()
    wt_d = nc.dram_tensor("wt", [KH, NF], BF16, kind="ExternalInput").ap()
    thr_d = nc.dram_tensor("thr", [P, 2 * CW3], F32, kind="ExternalInput").ap()
    sfc_d = nc.dram_tensor("sfc", [P, 9 * NOUT], BF16, kind="ExternalInput").ap()
    fcb_d = nc.dram_tensor("fcb", [NOUT, 1], F32, kind="ExternalInput").ap()
    out_d = nc.dram_tensor("out", [NOUT, B], F32, kind="ExternalOutput").ap()

    with tile.TileContext(nc) as tc, ExitStack() as ctx:
        const = ctx.enter_context(tc.tile_pool(name="const", bufs=1))
        imp = ctx.enter_context(tc.tile_pool(name="imp", bufs=1))
        zqp = ctx.enter_context(tc.tile_pool(name="zq", bufs=7))
        yp = ctx.enter_context(tc.tile_pool(name="y", bufs=6))
        gp = ctx.enter_context(tc.tile_pool(name="g", bufs=6))
        ttp = ctx.enter_context(tc.tile_pool(name="tt", bufs=1))

        wt = const.tile([KH, NF], BF16, tag="wt")
        nc.scalar.dma_start(wt[:], wt_d)
        thr = const.tile([P, 2 * CW3], F32, tag="thr")
        nc.scalar.dma_start(thr[:], thr_d)
        sfc = const.tile([P, 9 * NOUT], BF16, tag="sfc")
        nc.scalar.dma_start(sfc[:], sfc_d)
        fcb = const.tile([NOUT, 1], F32, tag="fcb")
        nc.scalar.dma_start(fcb[:], fcb_d)

        # per-batch-tile im2col tiles: a conv matmul depends only on its
        # own chunk's DMA, so the first tile starts as soon as chunk 0 lands
        imh_t = [imp.tile([KH, HO * P], BF16, tag=f"imh{bt}", name=f"imh{bt}")
                 for bt in range(BT)]
        iml_t = [imp.tile([KL, HO * P], BF16, tag=f"iml{bt}", name=f"iml{bt}")
                 for bt in range(BT)]
        G0 = 4 * P                   # first h-group of bt 0
        nc.sync.dma_start(imh_t[0][:, 0:G0], imh_d[:, 0:G0])
        nc.sync.dma_start(iml_t[0][:, 0:G0], iml_d[:, 0:G0])
        nc.sync.dma_start(imh_t[0][:, G0:HO * P], imh_d[:, G0:HO * P])
        nc.sync.dma_start(iml_t[0][:, G0:HO * P], iml_d[:, G0:HO * P])
        for bt in range(1, BT):
            s = bt * HO * P
            e = (bt + 1) * HO * P
            nc.sync.dma_start(imh_t[bt][:, :], imh_d[:, s:e])
            nc.sync.dma_start(iml_t[bt][:, :], iml_d[:, s:e])

        tT = [ttp.tile([P, B], BF16, tag=f"tT{j}", name=f"tT{j}") for j in range(9)]
        # persistent t' staging tiles, pad columns zeroed once
        tst = [ttp.tile([P, 3 * P], BF16, tag=f"ts{g}", name=f"ts{g}")
               for g in range(12)]
        for g in range(12):
            nc.vector.memset(tst[g][:, CW3:3 * P], 0.0)

        def conv_tile(bt, zp, fc_hook=None):
            zqs = {}
            for h in range(HO):
                k = bt * HO + h
                z = zp.tile([P, NF], F32, tag="z", name="z")
                for n0, n1 in ((0, 512), (512, 1024), (1024, NF)):
                    nc.tensor.matmul(z[:, n0:n1],
                                     lhsT=imh_t[bt][:, h * P:(h + 1) * P],
                                     rhs=wt[:, n0:n1],
                                     start=True, stop=False)
                    nc.tensor.matmul(z[:, n0:n1],
                                     lhsT=iml_t[bt][:, h * P:(h + 1) * P],
                                     rhs=wt[0:KL, n0:n1],
                                     start=False, stop=True)
                u = yp.tile([P, CW3], F32, tag="u", name="u")
                nc.vector.reduce_max(
                    u[:], z[:].rearrange("p (cw ww) -> p cw ww", ww=4),
                    axis=mybir.AxisListType.X)
                zqs[h % 4] = u
                if h % 4 == 3:
                    h3 = h // 4
                    m1 = yp.tile([P, CW3], F32, tag="m1", name="m1")
                    nc.vector.tensor_max(m1[:], zqs[0][:], zqs[1][:])
                    m2 = yp.tile([P, CW3], F32, tag="m2", name="m2")
                    nc.vector.tensor_max(m2[:], zqs[2][:], zqs[3][:])
                    m = gp.tile([P, CW3], F32, tag="m", name="m")
                    nc.vector.tensor_max(m[:], m1[:], m2[:])

                    g2 = gp.tile([P, 2 * CW3], BF16, tag="g2", name="g2")
                    nc.vector.tensor_tensor(
                        g2[:].rearrange("p (s c) -> p s c", s=2),
                        m[:].unsqueeze(1).broadcast_to([P, 2, CW3]),
                        thr[:].rearrange("p (s c) -> p s c", s=2),
                        ALU.is_gt)
                    t_ = tst[bt * 3 + h3]
                    nc.vector.tensor_add(t_[:, 0:CW3], g2[:, 0:CW3],
                                         g2[:, CW3:2 * CW3])
                    for ch in range(3):
                        eng = nc.scalar if ch == 1 else nc.sync
                        eng.dma_start_transpose(
                            tT[h3 * 3 + ch][:, bt * P:(bt + 1) * P],
                            t_[:, ch * P:(ch + 1) * P])
                    if fc_hook is not None:
                        fc_hook(h3)

        with tc.tile_pool(name="zp", bufs=2, space="PSUM") as zp:
            with tc.tile_pool(name="fcp", bufs=1, space="PSUM") as fcp:
                # two half-batch accumulators in separate banks: the bt0/bt1
                # half of the FC + output runs mid-kernel, shrinking the tail
                acc0 = fcp.tile([NOUT, B // 2], F32, tag="acc0")
                acc1 = fcp.tile([NOUT, B // 2], F32, tag="acc1")
                ob = const.tile([NOUT, B], F32, tag="ob")

                def fc_half(acc, c0):
                    for j in range(9):
                        kj = 32 if j % 3 == 2 else 128
                        nc.tensor.matmul(acc[:, :],
                                         lhsT=sfc[0:kj,
                                                  j * NOUT:(j + 1) * NOUT],
                                         rhs=tT[j][0:kj, c0:c0 + B // 2],
                                         start=(j == 0), stop=(j == 8))
                    nc.scalar.activation(ob[:, c0:c0 + B // 2], acc[:],
                                         mybir.ActivationFunctionType.Identity,
                                         bias=fcb[0:NOUT, 0:1], scale=1.0)
                    nc.sync.dma_start(out_d[:, c0:c0 + B // 2],
                                      ob[:, c0:c0 + B // 2])

                conv_tile(0, zp)
                conv_tile(1, zp)
                conv_tile(2, zp,
                          fc_hook=lambda h3: fc_half(acc0, 0) if h3 == 0
                          else None)
                conv_tile(3, zp)
                fc_half(acc1, B // 2)

    nc.compile()
    return nc


_NC_CACHE = None
LAST_RESULTS = None


def kernel(x, conv_w, conv_b, bn_gamma, bn_beta, bn_mean, bn_var, fc_w, fc_b):
    global _NC_CACHE, LAST_RESULTS
    x = np.asarray(x, np.float32).reshape(BFULL, H * W)
    wt, thr, sfc, fcb = _host_prep(
        np.asarray(conv_w, np.float64), np.asarray(conv_b, np.float64),
        np.asarray(bn_gamma, np.float64), np.asarray(bn_beta, np.float64),
        np.asarray(bn_mean, np.float64), np.asarray(bn_var, np.float64),
        np.asarray(fc_w, np.float32), np.asarray(fc_b, np.float64))

    if _NC_CACHE is None:
        _NC_CACHE = _build()
    nc = _NC_CACHE

    in_maps = []
    for i in range(NCORES):
        imh, iml = _host_im2col(x[i * B:(i + 1) * B])
        in_maps.append(dict(imh=imh, iml=iml, wt=wt, thr=thr, sfc=sfc, fcb=fcb))
    trace = _os.environ.get("KTRACE", "0") == "1"
    res = run_bass_kernel_spmd(nc, in_maps, core_ids=list(range(NCORES)),
                               trace=trace)
    LAST_RESULTS = res
    out = np.concatenate(
        [np.ascontiguousarray(res.results[i]["out"].T) for i in range(NCORES)],
        axis=0)
    return out.astype(np.float32)
